# revision 26
# baseline (speedup 1.0000x reference)
"""Trainium2 Bass kernel for nn_EB_19490561589325 (gnn_message_passing).

Reference computation (per batch element b, NP=4096 points, D=8 feats):
  - pairwise sq-distances via gram trick, self-distance masked to BIG
  - idx = 16 nearest neighbors per point, sorted ascending by distance
  - order-2/3 tuple features through tiny Linear+relu+mean(d) -> moments
  - second linear + relu, mean over tuples -> x_new; global mean -> z_new

Sharding: 8 cores = (batch b = core//2) x (row-half h = core%2); each core
handles 2048 rows x all 4096 candidate columns. Columns/gather source are
PERMUTED per core (own 2048 rows first) so the self-distance diagonal sits
at a core-independent position and the whole program is SPMD-uniform.

Device pipeline per 128-row tile:
  PE  : neg = 2*x_i.x_j - n2_i - n2_j (fp32 matmul, K=10, norm rows folded
        into the operands; -BIG*I accumulated onto the diagonal block)
  DVE : top-16 via max8 / max_index / match_replace (2 rounds), sorted
  DMA : 16 indirect gathers (one neighbor rank each) -> G_n[n, (t,d)] bf16
  PE  : transpose + 8 one-hot matmuls -> G_t2[t, (d,n)] (base partition 0)
  PE  : first layer vs host-built A matrix (bf16); ACT relu+bias eviction
  DVE : pairwise-tree sum over d -> moments
  PE  : second layer (block-diag [Wmx/120; Wvx/8], bf16) per tuple-slab pair
  ACT : relu+bias; GPSIMD accumulates x over tuples and z over everything
"""

import numpy as np
import itertools

import concourse.bass as bass
import concourse.bacc as bacc
import concourse.mybir as mybir
from concourse.tile import TileContext
from concourse.bass_utils import run_bass_kernel_spmd

try:
    from ml_dtypes import bfloat16 as np_bf16
except Exception:  # pragma: no cover
    import jax.numpy as _jnp

    np_bf16 = _jnp.bfloat16

F32 = mybir.dt.float32
BF16 = mybir.dt.bfloat16
I32 = mybir.dt.int32
U32 = mybir.dt.uint32

B, NP, D_FEAT, N_NBR = 4, 4096, 8, 16
ROWS_PER_CORE = NP // 2  # 2048
NTILES = ROWS_PER_CORE // 128  # 16
DM, DO, NM = 16, 8, 64
R_TUP = N_NBR - 1  # 15
NEG_BIG = -1.0e9
_CFG_OVERRIDE = {}


def _to_select(N, order):
    rows = [[0] + list(c) for c in itertools.combinations(range(1, N), order - 1)][: N - 1]
    return np.asarray(rows, dtype=np.int32)


_TS3 = _to_select(N_NBR, 3)


# ---------------------------------------------------------------- device code


def build_nc(cfg=None):
    cfg = dict(psD=2, psY_wide=False, psY=2, psG=2, ps2=2, tree_eng="vector", negb=2, gpb=2, ypb=3, r2b=3, spb=3, l2pair=False)
    if _CFG_OVERRIDE:
        cfg.update(_CFG_OVERRIDE)
    nc = bacc.Bacc()

    lm = nc.declare_dram_parameter("lm", [10, ROWS_PER_CORE + NP], F32, isOutput=False)
    xbp = nc.declare_dram_parameter("xbp", [NP, D_FEAT], BF16, isOutput=False)
    acat = nc.declare_dram_parameter("acat", [16, 1024], BF16, isOutput=False)
    b1 = nc.declare_dram_parameter("b1", [128, 8], F32, isOutput=False)
    w2 = nc.declare_dram_parameter("w2", [64, 72], BF16, isOutput=False)
    c2 = nc.declare_dram_parameter("c2", [72, 1], F32, isOutput=False)
    bigi = nc.declare_dram_parameter("bigi", [128, 256], F32, isOutput=False)
    pident = nc.declare_dram_parameter("pident", [128, 256], BF16, isOutput=False)
    xout = nc.declare_dram_parameter("xout", [8, ROWS_PER_CORE], F32, isOutput=True)
    zout = nc.declare_dram_parameter("zout", [64, 1], F32, isOutput=True)

    Relu = mybir.ActivationFunctionType.Relu
    Copy = mybir.ActivationFunctionType.Copy

    with TileContext(nc) as tc:
        with (
            tc.tile_pool(name="consts", bufs=1) as cp,
            tc.tile_pool(name="neg", bufs=cfg["negb"]) as negp,
            tc.tile_pool(name="small", bufs=cfg["spb"]) as sp,
            tc.tile_pool(name="gat", bufs=cfg["gpb"]) as gp,
            tc.tile_pool(name="ybuf", bufs=cfg["ypb"]) as yp,
            tc.tile_pool(name="mbuf", bufs=2) as mp,
            tc.tile_pool(name="r2buf", bufs=cfg["r2b"]) as r2p,
            tc.tile_pool(name="accp", bufs=1) as ap_,
            tc.tile_pool(name="psD", bufs=cfg["psD"], space="PSUM") as psD,
            tc.tile_pool(name="psY", bufs=cfg["psY"], space="PSUM") as psY,
            tc.tile_pool(name="psG", bufs=cfg["psG"], space="PSUM") as psG,
            tc.tile_pool(name="ps2", bufs=cfg["ps2"], space="PSUM") as ps2,
        ):
            lm_sb = cp.tile([10, ROWS_PER_CORE + NP], F32)
            acat_sb = cp.tile([16, 1024], BF16)
            b1_sb = cp.tile([128, 8], F32)
            w2_sb = cp.tile([128, 72], BF16)  # w2 duplicated in both halves
            c2_sb = cp.tile([72, 1], F32)
            bigi_sb = cp.tile([128, 256], F32)
            pident_sb = cp.tile([128, 256], BF16)
            nc.sync.dma_start(out=lm_sb[:], in_=lm[:])
            nc.sync.dma_start(out=acat_sb[:], in_=acat[:])
            nc.sync.dma_start(out=b1_sb[:], in_=b1[:])
            nc.sync.dma_start(out=w2_sb[0:64, :], in_=w2[:])
            nc.sync.dma_start(out=w2_sb[64:128, :], in_=w2[:])
            nc.sync.dma_start(out=c2_sb[:], in_=c2[:])
            nc.sync.dma_start(out=bigi_sb[:], in_=bigi[:])
            nc.sync.dma_start(out=pident_sb[:], in_=pident[:])

            # z accumulated across all tiles/slabs; x staging reused per tile
            zaccbig = ap_.tile([72, 256], F32)
            nc.vector.memzero(zaccbig[:])

            for t in range(NTILES):
                # ---- distances: neg = 2*x_i.x_j - n2_i - n2_j  (fp32)
                neg = negp.tile([128, NP], F32)
                diag_c8, diag_off = (t * 128) // 512, (t * 128) % 512
                for c8 in range(8):
                    pd = psD.tile([128, 512], F32)
                    nc.tensor.matmul(
                        pd[:],
                        lhsT=lm_sb[:, t * 128 : t * 128 + 128],
                        rhs=lm_sb[:, ROWS_PER_CORE + c8 * 512 : ROWS_PER_CORE + c8 * 512 + 512],
                        start=True,
                        stop=True,
                    )
                    if c8 == diag_c8:
                        # accumulate -BIG*I onto the self-distance diagonal
                        nc.tensor.matmul(
                            pd[:, diag_off : diag_off + 128],
                            lhsT=bigi_sb[:, 0:128],
                            rhs=bigi_sb[:, 128:256],
                            start=False,
                            stop=True,
                            skip_group_check=True,
                        )
                    nc.scalar.activation(neg[:, c8 * 512 : c8 * 512 + 512], pd[:], Copy)

                # ---- top-16 (largest neg == smallest distance), sorted
                m8a = sp.tile([128, 8], F32, tag="m8a")
                m8b = sp.tile([128, 8], F32, tag="m8b")
                idx16 = sp.tile([128, 16], U32, tag="idx16")
                nc.vector.max(out=m8a[:], in_=neg[:])
                nc.vector.max_index(out=idx16[:, 0:8], in_max=m8a[:], in_values=neg[:])
                nc.vector.match_replace(
                    out=neg[:], in_to_replace=m8a[:], in_values=neg[:], imm_value=NEG_BIG
                )
                nc.vector.max(out=m8b[:], in_=neg[:])
                nc.vector.max_index(out=idx16[:, 8:16], in_max=m8b[:], in_values=neg[:])

                # ---- gather neighbor features: G_n[n, (t,d)] bf16
                g_n = gp.tile([128, 128], BF16, tag="gn")
                for tt in range(N_NBR):
                    nc.gpsimd.indirect_dma_start(
                        out=g_n[:, tt * 8 : tt * 8 + 8],
                        out_offset=None,
                        in_=xbp[:],
                        in_offset=bass.IndirectOffsetOnAxis(
                            ap=idx16[:, tt : tt + 1], axis=0
                        ),
                    )

                # ---- layout fix: G_n -> P1[(t,d), n] -> G_t2[t, (d,n)]
                p1 = psG.tile([128, 128], BF16, tag="pg")
                nc.tensor.transpose(out=p1[:], in_=g_n[:], identity=pident_sb[:, 0:128])
                p1s = gp.tile([128, 128], BF16, tag="p1s")
                nc.scalar.activation(p1s[:], p1[:], Copy)
                g_t2 = gp.tile([16, 1024], BF16, tag="gt2")
                for half in range(2):
                    pg = psG.tile([16, 512], F32, tag="pg")
                    for dj in range(4):
                        d = half * 4 + dj
                        nc.tensor.matmul(
                            pg[:, dj * 128 : dj * 128 + 128],
                            lhsT=pident_sb[:, 128 + d * 16 : 128 + d * 16 + 16],
                            rhs=p1s[:],
                            start=True,
                            stop=True,
                        )
                    nc.scalar.activation(
                        g_t2[:, half * 512 : half * 512 + 512], pg[:], Copy
                    )

                # ---- first layer: y[rkk, (d,n)] = A.T @ G, relu+bias, sum d
                mbig = mp.tile([128, 1024], BF16)
                tree = nc.gpsimd if cfg["tree_eng"] == "gpsimd" else nc.vector
                for mc in range(8):
                    ysb = yp.tile([128, 1024], BF16, tag="ysb")
                    if cfg["psY_wide"]:
                        pY = psY.tile([128, 1024], F32, tag="pY")
                        for half in range(2):
                            nc.tensor.matmul(
                                pY[:, half * 512 : half * 512 + 512],
                                lhsT=acat_sb[:, mc * 128 : mc * 128 + 128],
                                rhs=g_t2[:, half * 512 : half * 512 + 512],
                                start=True,
                                stop=True,
                            )
                        nc.scalar.activation(
                            ysb[:], pY[:], Relu, bias=b1_sb[:, mc : mc + 1]
                        )
                    else:
                        for half in range(2):
                            pY = psY.tile([128, 512], F32, tag="pY")
                            nc.tensor.matmul(
                                pY[:],
                                lhsT=acat_sb[:, mc * 128 : mc * 128 + 128],
                                rhs=g_t2[:, half * 512 : half * 512 + 512],
                                start=True,
                                stop=True,
                            )
                            nc.scalar.activation(
                                ysb[:, half * 512 : half * 512 + 512],
                                pY[:],
                                Relu,
                                bias=b1_sb[:, mc : mc + 1],
                            )
                    # pairwise-tree sum over the 8 d-planes (cols are d-major)
                    tr1 = yp.tile([128, 512], BF16, tag="tr1")
                    tree.tensor_add(tr1[:], ysb[:, 0:512], ysb[:, 512:1024])
                    tr2 = yp.tile([128, 256], BF16, tag="tr2")
                    tree.tensor_add(tr2[:], tr1[:, 0:256], tr1[:, 256:512])
                    tree.tensor_add(
                        mbig[:, mc * 128 : mc * 128 + 128],
                        tr2[:, 0:128],
                        tr2[:, 128:256],
                    )

                # ---- second layer per tuple-slab pair
                accx = sp.tile([8, 128], F32, tag="accx")
                if cfg["l2pair"]:
                    for qp in range(4):
                        widths = (256, 128) if qp == 3 else (256, 256)
                        p2 = ps2.tile([72, 512], F32, tag="p2")
                        for j in range(2):
                            rows0 = 0 if j == 0 else 64
                            nc.tensor.matmul(
                                p2[:, j * 256 : j * 256 + widths[j]],
                                lhsT=w2_sb[rows0 : rows0 + 64, :],
                                rhs=mbig[rows0 : rows0 + 64, 256 * qp : 256 * qp + widths[j]],
                                start=True,
                                stop=True,
                            )
                        wtot = widths[0] + widths[1]
                        r2 = r2p.tile([72, 512], BF16, tag="r2")
                        nc.scalar.activation(r2[:, :wtot], p2[:, :wtot], Relu, bias=c2_sb[:])
                        for cc in range(wtot // 128):
                            if qp == 0 and cc == 0:
                                nc.gpsimd.tensor_copy(accx[:], r2[0:8, 0:128])
                            else:
                                nc.gpsimd.tensor_add(
                                    accx[:], accx[:], r2[0:8, cc * 128 : cc * 128 + 128]
                                )
                        nc.gpsimd.tensor_add(
                            zaccbig[:, 0:wtot // 2], zaccbig[:, 0:wtot // 2], r2[:, 0:wtot // 2]
                        )
                        nc.gpsimd.tensor_add(
                            zaccbig[:, 0:wtot - wtot // 2], zaccbig[:, 0:wtot - wtot // 2],
                            r2[:, wtot // 2:wtot]
                        )
                else:
                    for q in range(8):
                        rows0 = 0 if q < 4 else 64
                        c0 = 256 * (q % 4)
                        w = 128 if q == 7 else 256
                        p2 = ps2.tile([72, 256], F32, tag="p2")
                        nc.tensor.matmul(
                            p2[:, :w],
                            lhsT=w2_sb[rows0 : rows0 + 64, :],
                            rhs=mbig[rows0 : rows0 + 64, c0 : c0 + w],
                            start=True,
                            stop=True,
                        )
                        r2 = r2p.tile([72, 256], BF16, tag="r2")
                        nc.scalar.activation(r2[:, :w], p2[:, :w], Relu, bias=c2_sb[:])
                        for cc in range(w // 128):
                            if q == 0 and cc == 0:
                                nc.gpsimd.tensor_copy(accx[:], r2[0:8, 0:128])
                            else:
                                nc.gpsimd.tensor_add(
                                    accx[:], accx[:], r2[0:8, cc * 128 : cc * 128 + 128]
                                )
                        nc.gpsimd.tensor_add(
                            zaccbig[:, 0:w], zaccbig[:, 0:w], r2[:, 0:w]
                        )

                nc.sync.dma_start(out=xout[:, t * 128 : t * 128 + 128], in_=accx[:])

            zrow = ap_.tile([72, 1], F32)
            nc.vector.tensor_reduce(
                out=zrow[:], in_=zaccbig[:], axis=mybir.AxisListType.X,
                op=mybir.AluOpType.add,
            )
            nc.sync.dma_start(out=zout[:], in_=zrow[8:72, :])

    nc.compile()
    return nc


# ---------------------------------------------------------------- host prep


def build_shared_inputs(Wm2, bm2, Wm3, bm3, Wv2, bv2, Wv3, bv3, Wmx, Wvx):
    """A matrices, first-layer bias layout, second-layer block weights."""
    A = np.zeros((16, 1024), dtype=np.float32)
    bias_all = np.zeros(1024, dtype=np.float32)
    for r in range(R_TUP):
        base = 64 * r
        A[0, base : base + 16] += Wm2[0]
        A[r + 1, base : base + 16] += Wm2[1]
        bias_all[base : base + 16] = bm2
        t0, t1, t2 = _TS3[r]
        q0 = base + 16
        A[t0, q0 : q0 + 16] += Wm3[0]
        A[t1, q0 : q0 + 16] += Wm3[1]
        A[t2, q0 : q0 + 16] += Wm3[2]
        bias_all[q0 : q0 + 16] = bm3
        q0 = base + 32
        A[0, q0 : q0 + 16] += Wv2[0]
        A[r + 1, q0 : q0 + 16] += Wv2[1]
        bias_all[q0 : q0 + 16] = bv2
        q0 = base + 48
        A[t0, q0 : q0 + 16] += Wv3[0]
        A[t1, q0 : q0 + 16] += Wv3[1]
        A[t2, q0 : q0 + 16] += Wv3[2]
        bias_all[q0 : q0 + 16] = bv3

    b1 = np.ascontiguousarray(bias_all.reshape(8, 128).T).astype(np.float32)

    # second layer: x-side absorbs mean over d (/8) and tuples (/15)
    w2 = np.zeros((64, 72), dtype=np.float32)
    w2[0:32, 0:8] = Wmx / (8.0 * R_TUP)
    w2[32:64, 8:72] = Wvx / 8.0

    # pident: [identity(bf16) | perm matrix moving (t,d)->(d,t) slabs]
    permT = np.zeros((128, 128), dtype=np.float32)
    for tt in range(16):
        for d in range(8):
            permT[tt * 8 + d, d * 16 + tt] = 1.0
    pident = np.concatenate([np.eye(128, dtype=np.float32), permT], axis=1)

    return {
        "acat": A.astype(np_bf16),
        "b1": b1,
        "w2": w2.astype(np_bf16),
        "bigi": np.concatenate(
            [np.eye(128, dtype=np.float32) * NEG_BIG, np.eye(128, dtype=np.float32)],
            axis=1,
        ),
        "pident": pident.astype(np_bf16),
    }


def build_core_inputs(core, x3, z, Wmz, bmz, bmx, Wvz, bvz, bvx, shared):
    b, h = core // 2, core % 2
    xb = x3[b]  # (4096, 8) f32
    own = np.arange(h * ROWS_PER_CORE, (h + 1) * ROWS_PER_CORE)
    other = np.arange((1 - h) * ROWS_PER_CORE, (2 - h) * ROWS_PER_CORE)
    perm = np.concatenate([own, other])
    xp = np.ascontiguousarray(xb[perm])  # (4096, 8)

    n2 = (xp * xp).sum(axis=1, dtype=np.float32)
    ones = np.ones(NP, dtype=np.float32)
    M = np.concatenate([xp.T, -n2[None, :], ones[None, :]], axis=0).astype(np.float32)
    L = np.concatenate([2.0 * xp.T, ones[None, :], -n2[None, :]], axis=0).astype(
        np.float32
    )[:, :ROWS_PER_CORE]

    c2 = np.zeros((72, 1), dtype=np.float32)
    c2[0:8, 0] = (bmx + z[b] @ Wmz + bmz) / R_TUP
    c2[8:72, 0] = bvx + z[b] @ Wvz + bvz

    m = {
        "lm": np.ascontiguousarray(np.concatenate([L, M], axis=1)),
        "xbp": xp.astype(np_bf16),
        "c2": c2,
    }
    m.update(shared)
    return m


_NC_CACHE = {}


def kernel(**inputs):
    x = np.asarray(inputs["x"], dtype=np.float32)
    z = np.asarray(inputs["z"], dtype=np.float32)
    x3 = x.reshape(B, NP, D_FEAT)

    shared = build_shared_inputs(
        np.asarray(inputs["Wm2"], np.float32), np.asarray(inputs["bm2"], np.float32),
        np.asarray(inputs["Wm3"], np.float32), np.asarray(inputs["bm3"], np.float32),
        np.asarray(inputs["Wv2"], np.float32), np.asarray(inputs["bv2"], np.float32),
        np.asarray(inputs["Wv3"], np.float32), np.asarray(inputs["bv3"], np.float32),
        np.asarray(inputs["Wmx"], np.float32), np.asarray(inputs["Wvx"], np.float32),
    )
    in_maps = [
        build_core_inputs(
            c, x3, z,
            np.asarray(inputs["Wmz"], np.float32), np.asarray(inputs["bmz"], np.float32),
            np.asarray(inputs["bmx"], np.float32),
            np.asarray(inputs["Wvz"], np.float32), np.asarray(inputs["bvz"], np.float32),
            np.asarray(inputs["bvx"], np.float32),
            shared,
        )
        for c in range(8)
    ]

    if "nc" not in _NC_CACHE:
        _NC_CACHE["nc"] = build_nc()
    nc = _NC_CACHE["nc"]

    res = run_bass_kernel_spmd(nc, in_maps, list(range(8)))

    x_new = np.empty((B, NP, DO), dtype=np.float32)
    z_new = np.empty((B, NM), dtype=np.float32)
    for bb in range(B):
        lo = res.results[2 * bb]
        hi = res.results[2 * bb + 1]
        x_new[bb, :ROWS_PER_CORE] = lo["xout"].T
        x_new[bb, ROWS_PER_CORE:] = hi["xout"].T
        z_new[bb] = (lo["zout"][:, 0] + hi["zout"][:, 0]) / float(NP * R_TUP)
    return x_new.reshape(B, NP * DO), z_new
